# revision 1
# baseline (speedup 1.0000x reference)
"""Trainium2 Bass kernel for EnhancedMambaMixer (B=2, L=1024, H=1024, D=2048, N=16, K=4, R=64).

Sharding: 8-way tensor-parallel over intermediate_size D (256 channels/core).
Each core computes its D-shard of in_proj/conv/scan and a partial out_proj;
a 786KB in-kernel AllReduce combines the x_proj partials (dt_lr/B/C are
reductions over the full D). Host sums the 8 out_proj partials.

Layout on-chip: channels in partitions, time fused as B*L=2048 in the free dim.

Engine plan (v2):
  PE    - in_proj (f32r), x_proj/dt matmuls, y = sum_n g_n via identity-matmul
          PSUM accumulation (bf16), out_proj (bf16)
  ACT   - SiLU, softplus (exp+ln, one act-table switch), the 32 exp(dt*-A_n),
          PSUM evictions
  DVE   - conv taps (stt), scans (2cyc/elem, DVE-only), g = h*C (bf16 2x)
  GPSIMD- dBu = dtx*B (bf16)
  DMA   - B/C row broadcasts replicated from DRAM (bf16)
"""

import ml_dtypes
import numpy as np

# Problem constants (hardcoded; kernel.py must be self-contained).
B, L, H = 2, 1024, 1024
D = 2048
N = 16
K = 4
R = 64
NCORES = 8
DP = D // NCORES          # 256 channels per core
T = B * L                 # 2048 fused time steps
TPAD = T + B * (K - 1)    # padded x for causal conv: [3 zeros][b0][3 zeros][b1]
NT = T // 512             # moving-dim tiles of 512

_CACHE = {}


def _build_module(dbu_on_gpsimd=False):
    import concourse.bacc as bacc
    import concourse.mybir as mybir
    import concourse.tile as tile

    f32 = mybir.dt.float32
    f32r = mybir.dt.float32r
    bf16 = mybir.dt.bfloat16
    Alu = mybir.AluOpType
    Act = mybir.ActivationFunctionType

    nc = bacc.Bacc(
        "TRN2",
        target_bir_lowering=False,
        debug=False,
        num_devices=NCORES,
    )

    # ---- I/O -------------------------------------------------------------
    hsT = nc.dram_tensor("hsT", [H, T], bf16, kind="ExternalInput").ap()
    winT = nc.dram_tensor("winT", [H, 2 * DP], bf16, kind="ExternalInput").ap()
    wxT = nc.dram_tensor("wxT", [DP, R + 2 * N], bf16, kind="ExternalInput").ap()
    wdtT = nc.dram_tensor("wdtT", [R, DP], f32, kind="ExternalInput").ap()
    bdt = nc.dram_tensor("bdt", [DP, 1], f32, kind="ExternalInput").ap()
    negA = nc.dram_tensor("negA", [DP, N], f32, kind="ExternalInput").ap()
    convb = nc.dram_tensor("convb", [DP, 1], f32, kind="ExternalInput").ap()
    dparam = nc.dram_tensor("dparam", [DP, 1], f32, kind="ExternalInput").ap()
    woutT = nc.dram_tensor("woutT", [DP, H], bf16, kind="ExternalInput").ap()
    eye_d = nc.dram_tensor("eye", [128, 128], bf16, kind="ExternalInput").ap()
    convdiag = nc.dram_tensor(
        "convdiag", [DP // 128 * K, 128, 128], bf16, kind="ExternalInput"
    ).ap()
    outT = [
        nc.dram_tensor(f"outT_part{i}", [H, T], bf16, kind="ExternalOutput").ap()
        for i in range(DP // 128)
    ]

    DT2 = DP // 128  # d-tiles per core (2)

    with tile.TileContext(nc) as tc:
        with (
            tc.tile_pool(name="persist", bufs=1) as pp,
            tc.tile_pool(name="dram", bufs=1, space="DRAM") as dp,
        ):
            # ---------------- persistent SBUF tiles ----------------------
            xpad = [pp.tile([128, TPAD], bf16, name=f"xpad{i}") for i in range(DT2)]
            x = [pp.tile([128, T], bf16, name=f"x{i}") for i in range(DT2)]
            sg_raw = [pp.tile([128, T], f32, name=f"sgr{i}") for i in range(DT2)]
            sg = [pp.tile([128, T], bf16, name=f"sg{i}") for i in range(DT2)]
            dt_t = [pp.tile([128, T], f32, name=f"dt{i}") for i in range(DT2)]
            dtx = [pp.tile([128, T], bf16, name=f"dtx{i}") for i in range(DT2)]
            yf = [pp.tile([128, T], bf16, name=f"yf{i}") for i in range(DT2)]
            dtlr_g = pp.tile([R, T], f32r, name="dtlr_g")
            eye_sb = pp.tile([128, 128], bf16, name="eye_sb")

            convdiag_sb = [
                [pp.tile([128, 128], bf16, name=f"cdiag{i}_{k}") for k in range(K)]
                for i in range(DT2)
            ]
            convb_sb = [pp.tile([128, 1], f32, name=f"convb_sb{i}") for i in range(DT2)]
            bdt_sb = [pp.tile([128, 1], f32, name=f"bdt_sb{i}") for i in range(DT2)]
            negA_sb = [pp.tile([128, N], f32, name=f"negA_sb{i}") for i in range(DT2)]
            dparam_sb = [
                pp.tile([128, 1], f32, name=f"dparam_sb{i}") for i in range(DT2)
            ]
            wxT_sb = [
                pp.tile([128, R + 2 * N], bf16, name=f"wxT_sb{i}") for i in range(DT2)
            ]
            wdtT_sb = pp.tile([R, DP], f32r, name="wdtT_sb")
            woutT_sb = [
                pp.tile([128, H], bf16, name=f"woutT_sb{i}") for i in range(DT2)
            ]

            for i in range(DT2):
                rs = slice(128 * i, 128 * (i + 1))
                for k in range(K):
                    nc.sync.dma_start(
                        convdiag_sb[i][k][:], convdiag[K * i + k, :, :]
                    )
                nc.sync.dma_start(convb_sb[i][:], convb[rs, :])
                nc.sync.dma_start(bdt_sb[i][:], bdt[rs, :])
                nc.sync.dma_start(negA_sb[i][:], negA[rs, :])
                nc.sync.dma_start(dparam_sb[i][:], dparam[rs, :])
                nc.sync.dma_start(wxT_sb[i][:], wxT[rs, :])
                nc.sync.dma_start(woutT_sb[i][:], woutT[rs, :])
            nc.sync.dma_start(wdtT_sb[:], wdtT.bitcast(f32r))
            nc.sync.dma_start(eye_sb[:], eye_d)
            for i in range(DT2):
                nc.gpsimd.memset(xpad[i][:, 0:3], 0.0)
                nc.gpsimd.memset(xpad[i][:, 1027:1030], 0.0)

            # ---------------- phase 1: in_proj ----------------------------
            KH = H // 128  # 8 contraction tiles
            p1_cm = tc.tile_pool(name="ph1", bufs=1)
            p1 = p1_cm.__enter__()
            ps1_cm = tc.tile_pool(name="ps1", bufs=4, space="PSUM")
            ps1 = ps1_cm.__enter__()
            hsT_sb = [p1.tile([128, T], bf16, name=f"hsT{k}") for k in range(KH)]
            winT_sb = [
                p1.tile([128, 2 * DP], bf16, name=f"winT{k}") for k in range(KH)
            ]
            for k in range(KH):
                nc.sync.dma_start(hsT_sb[k][:], hsT[128 * k : 128 * (k + 1), :])
                nc.sync.dma_start(winT_sb[k][:], winT[128 * k : 128 * (k + 1), :])

            def in_proj_group(m, evict):
                for t in range(NT):
                    pj = ps1.tile([128, 512], f32, name="pj", tag="pj", bufs=4)
                    for k in range(KH):
                        nc.tensor.matmul(
                            pj[:],
                            winT_sb[k][:, 128 * m : 128 * (m + 1)],
                            hsT_sb[k][:, 512 * t : 512 * (t + 1)],
                            start=(k == 0),
                            stop=(k == KH - 1),
                        )
                    evict(t, pj)

            def evict_x(m):
                def f(t, pj):
                    # pre-conv x -> padded layout (3-col zero pad per batch)
                    dst = 3 + 512 * t if t < 2 else 1030 + 512 * (t - 2)
                    nc.scalar.copy(xpad[m][:, dst : dst + 512], pj[:])
                return f

            def evict_gate(m):
                def f(t, pj):
                    nc.scalar.copy(sg_raw[m][:, 512 * t : 512 * (t + 1)], pj[:])
                return f

            # x-half first: unblocks conv -> x_proj -> AllReduce asap
            for m in range(DT2):
                in_proj_group(m, evict_x(m))

            # ---------------- phase 2: depthwise causal conv on PE --------
            # per tap k: accumulate diag(w_k) @ xpad[:, shifted] into PSUM,
            # then silu(psum + conv_b) evicts to x (bf16)
            ps3_cm = tc.tile_pool(name="ps3", bufs=1, space="PSUM")
            ps3 = ps3_cm.__enter__()
            for i in range(DT2):
                cps = ps3.tile([128, T], f32, name=f"cps{i}", tag="sp", bufs=1)
                for nt in range(NT):
                    base = 0 if nt < 2 else 1027
                    col = base + 512 * (nt % 2)
                    for k in range(K):
                        nc.tensor.matmul(
                            cps[:, 512 * nt : 512 * (nt + 1)],
                            convdiag_sb[i][k][:],
                            xpad[i][:, col + k : col + k + 512],
                            start=(k == 0),
                            stop=(k == K - 1),
                        )
                # x = silu(xconv + conv_b)
                nc.scalar.activation(x[i][:], cps[:], Act.Silu, bias=convb_sb[i][:])

            # gate m=0 keeps the PE busy while conv/silu run on DVE/ACT
            def gate_group(m):
                for t in range(NT):
                    pj = ps1.tile([128, 512], f32, name="pjb", tag="pj", bufs=4)
                    for k in range(KH):
                        nc.tensor.matmul(
                            pj[:],
                            winT_sb[k][:, 128 * (DT2 + m) : 128 * (DT2 + m + 1)],
                            hsT_sb[k][:, 512 * t : 512 * (t + 1)],
                            start=(k == 0),
                            stop=(k == KH - 1),
                        )
                    nc.scalar.copy(sg_raw[m][:, 512 * t : 512 * (t + 1)], pj[:])

            # ---------------- phase 3: x_proj partial + AllReduce ---------
            sp_ps = ps3.tile([96, T], f32, name="sp_ps", tag="sp", bufs=1)
            for t in range(NT):
                for kd in range(DT2):
                    nc.tensor.matmul(
                        sp_ps[:, 512 * t : 512 * (t + 1)],
                        wxT_sb[kd][:],
                        x[kd][:, 512 * t : 512 * (t + 1)],
                        start=(kd == 0),
                        stop=(kd == DT2 - 1),
                    )
            ssm_local = pp.tile([96, T], f32, name="ssm_local")
            nc.scalar.copy(ssm_local[:], sp_ps[:])

            ar_in = dp.tile([96, T], f32, name="ar_in")
            ar_out = dp.tile([96, T], f32, name="ar_out", addr_space="Shared")
            nc.sync.dma_start(ar_in[:], ssm_local[:])
            nc.gpsimd.collective_compute(
                "AllReduce",
                Alu.add,
                replica_groups=[list(range(NCORES))],
                ins=[ar_in[:]],
                outs=[ar_out[:]],
            )
            nc.sync.dma_start(dtlr_g[:], ar_out[0:R, :].bitcast(f32r))

            # both gate halves fill the PE while the AllReduce is in flight
            gate_group(0)
            gate_group(1)

            # B/C rows -> bf16 DRAM via one casting DMA (gpsimd can cast)
            bc_dram = dp.tile([2 * N, T], bf16, name="bc_dram")
            nc.gpsimd.dma_start(bc_dram[:], ar_out[R : R + 2 * N, :])

            # ---------------- phase 4: dt = softplus(W_dt @ dt_lr + b) ----
            for m in range(DT2):
                dt_ps = ps3.tile([128, T], f32, name="dt_ps", tag="sp", bufs=1)
                for t in range(NT):
                    nc.tensor.matmul(
                        dt_ps[:, 512 * t : 512 * (t + 1)],
                        wdtT_sb[:, 128 * m : 128 * (m + 1)],
                        dtlr_g[:, 512 * t : 512 * (t + 1)],
                        start=True,
                        stop=True,
                    )
                # softplus(z) = ln(exp(z) + 1); keeps ACT in the ln+exp table
                nc.scalar.activation(dt_t[m][:], dt_ps[:], Act.Exp, bias=bdt_sb[m][:])
                nc.scalar.activation(dt_t[m][:], dt_t[m][:], Act.Ln, bias=1.0)
                nc.vector.tensor_mul(dtx[m][:], dt_t[m][:], x[m][:])
                nc.gpsimd.memset(dt_t[m][:, 1024:1025], 1.0e9)
            ps3_cm.__exit__(None, None, None)
            ps1_cm.__exit__(None, None, None)
            p1_cm.__exit__(None, None, None)

            # ---------------- phase 5+6: per-dtile scan + out_proj --------
            # dtile-outer; dtile0's out_proj chunks are interleaved into
            # dtile1's n-loop (after each exp, so the ACT queue never blocks
            # the scan chain); the last dtile's out_proj pipelines per
            # 512-column tile right behind the final identity-matmuls.
            with (
                tc.tile_pool(name="loop", bufs=2) as lp,
                tc.tile_pool(name="psY", bufs=1, space="PSUM") as psy,
                tc.tile_pool(name="ps6", bufs=4, space="PSUM") as ps6,
            ):
                def out_chunk(i, m, t, evict_dve=False):
                    sl = slice(512 * t, 512 * (t + 1))
                    po = ps6.tile([128, 512], f32, name="po", tag="po", bufs=4)
                    nc.tensor.matmul(
                        po[:],
                        woutT_sb[i][:, 128 * m : 128 * (m + 1)],
                        yf[i][:, sl],
                        start=True,
                        stop=True,
                    )
                    ot = lp.tile([128, 512], bf16, name="ot", tag="ot", bufs=4)
                    if evict_dve:
                        nc.vector.tensor_copy(ot[:], po[:])
                    else:
                        nc.scalar.copy(ot[:], po[:])
                    nc.sync.dma_start(outT[i][128 * m : 128 * (m + 1), sl], ot[:])

                for i in range(DT2):
                    y_ps = psy.tile([128, T], f32, name=f"y_ps{i}", tag="y", bufs=1)
                    for n in range(N):
                        Bb = lp.tile([128, T], bf16, name="Bb", tag="Bb", bufs=2)
                        Cb = lp.tile([128, T], bf16, name="Cb", tag="Cb", bufs=2)
                        nc.sync.dma_start(
                            Bb[:], bc_dram[n : n + 1, :].to_broadcast([128, T])
                        )
                        nc.sync.dma_start(
                            Cb[:], bc_dram[N + n : N + n + 1, :].to_broadcast([128, T])
                        )
                        dA = lp.tile([128, T], f32, name="dA", tag="dA", bufs=3)
                        dBu = lp.tile([128, T], bf16, name="dBu", tag="dBu")
                        h = lp.tile([128, T], bf16, name="h", tag="h")
                        g = lp.tile([128, T], bf16, name="g", tag="g")
                        nc.scalar.activation(
                            dA[:],
                            dt_t[i][:],
                            Act.Exp,
                            scale=negA_sb[i][:, n : n + 1],
                        )
                        nc.vector.tensor_tensor(
                            out=dBu[:], in0=dtx[i][:], in1=Bb[:], op=Alu.mult
                        )
                        last = n == N - 1
                        if not last:
                            nc.vector.tensor_tensor_scan(
                                h[:], dA[:], dBu[:], 0.0, Alu.mult, Alu.add
                            )
                            nc.vector.tensor_mul(g[:], h[:], Cb[:])
                            for t in range(NT):
                                nc.tensor.matmul(
                                    y_ps[:, 512 * t : 512 * (t + 1)],
                                    eye_sb[:],
                                    g[:, 512 * t : 512 * (t + 1)],
                                    start=(n == 0),
                                    stop=False,
                                )
                        else:
                            # final iteration: chain 4 scan chunks so the
                            # g/y/gating/out_proj tail pipelines per column
                            # tile instead of waiting for the full scan
                            for t in range(NT):
                                sl = slice(512 * t, 512 * (t + 1))
                                nc.vector.tensor_tensor_scan(
                                    h[:, sl],
                                    dA[:, sl],
                                    dBu[:, sl],
                                    0.0 if t == 0 else h[:, 512 * t - 1 : 512 * t],
                                    Alu.mult,
                                    Alu.add,
                                )
                                nc.vector.tensor_mul(g[:, sl], h[:, sl], Cb[:, sl])
                                nc.tensor.matmul(
                                    y_ps[:, sl],
                                    eye_sb[:],
                                    g[:, sl],
                                    start=False,
                                    stop=True,
                                )
                        if i > 0:
                            # previous dtile's out_proj, two chunks per n
                            for j in (2 * n, 2 * n + 1):
                                out_chunk(i - 1, j // NT, j % NT)

                    # ---- gate: yf = (y + x*D) * silu(gate), bf16 ---------
                    nc.scalar.activation(sg[i][:], sg_raw[i][:], Act.Silu)
                    tmp = lp.tile([128, T], bf16, name="tmp", tag="tmp")
                    for t in range(NT):
                        sl = slice(512 * t, 512 * (t + 1))
                        nc.vector.scalar_tensor_tensor(
                            tmp[:, sl],
                            x[i][:, sl],
                            dparam_sb[i][:],
                            y_ps[:, sl],
                            Alu.mult,
                            Alu.add,
                        )
                        nc.vector.tensor_mul(yf[i][:, sl], tmp[:, sl], sg[i][:, sl])
                        if i == DT2 - 1:
                            for m in range(H // 128):
                                out_chunk(i, m, t, evict_dve=(m % 2 == 1))

    nc.compile()
    return nc


def _get_module():
    if "nc" not in _CACHE:
        _CACHE["nc"] = _build_module()
    return _CACHE["nc"]


def _conv_diag(cw):
    out = np.zeros((DP // 128 * K, 128, 128), dtype=np.float32)
    for i in range(DP // 128):
        for k in range(K):
            np.fill_diagonal(out[K * i + k], cw[128 * i : 128 * (i + 1), k])
    return out.astype(ml_dtypes.bfloat16)


def _shard_inputs(inputs):
    """Build the 8 per-core input maps (host-side transposes are free)."""
    hs = np.asarray(inputs["hidden_states"], dtype=np.float32)
    W_in = np.asarray(inputs["W_in"], dtype=np.float32)
    conv_w = np.asarray(inputs["conv_w"], dtype=np.float32)
    conv_b = np.asarray(inputs["conv_b"], dtype=np.float32)
    W_x = np.asarray(inputs["W_x"], dtype=np.float32)
    W_dt = np.asarray(inputs["W_dt"], dtype=np.float32)
    b_dt = np.asarray(inputs["b_dt"], dtype=np.float32)
    A_log = np.asarray(inputs["A_log"], dtype=np.float32)
    D_param = np.asarray(inputs["D_param"], dtype=np.float32)
    W_out = np.asarray(inputs["W_out"], dtype=np.float32)

    hsT = np.ascontiguousarray(hs.reshape(T, H).T)
    in_maps = []
    for c in range(NCORES):
        dc = slice(DP * c, DP * (c + 1))
        winT = np.ascontiguousarray(
            np.concatenate([W_in[dc], W_in[D + DP * c : D + DP * (c + 1)]], axis=0).T
        )
        in_maps.append(
            {
                "hsT": hsT.astype(ml_dtypes.bfloat16),
                "eye": np.eye(128, dtype=np.float32).astype(ml_dtypes.bfloat16),
                "winT": winT.astype(ml_dtypes.bfloat16),
                "wxT": np.ascontiguousarray(W_x[:, dc].T).astype(ml_dtypes.bfloat16),
                "wdtT": np.ascontiguousarray(W_dt[dc].T),
                "bdt": np.ascontiguousarray(b_dt[dc][:, None]),
                "negA": np.ascontiguousarray(-np.exp(A_log[dc])),
                "convdiag": _conv_diag(conv_w[dc, 0, :]),
                "convb": np.ascontiguousarray(conv_b[dc][:, None]),
                "dparam": np.ascontiguousarray(D_param[dc][:, None]),
                "woutT": np.ascontiguousarray(W_out[:, dc].T).astype(
                    ml_dtypes.bfloat16
                ),
            }
        )
    return in_maps


def kernel(**inputs):
    from concourse import bass_utils

    nc = _get_module()
    in_maps = _shard_inputs(inputs)
    res = bass_utils.run_bass_kernel_spmd(
        nc, in_maps, core_ids=list(range(NCORES))
    )
    _CACHE["last_results"] = res
    acc = np.zeros((H, T), dtype=np.float32)
    for r in res.results:
        acc += r["outT_part0"].astype(np.float32)
        acc += r["outT_part1"].astype(np.float32)
    return np.ascontiguousarray(acc.T).reshape(B, L, H)

